# revision 31
# baseline (speedup 1.0000x reference)
"""Causal self-attention (B=2, T=2048, C=1024, H=16, RoPE) on 8 TRN2 NeuronCores.

Sharding: core i handles batch b = i//4 and head group g = i%4 (4 heads each).
Each core computes q/k (transposed, RoPE'd), v, causal attention, and a partial
output projection; the host sums the 4 partials per batch element (tensor-
parallel unshard) and adds the constant term b_proj + b_v @ W_proj, which is
independent of x because softmax rows sum to 1.

Layout strategy (no on-chip transposes):
  - host passes x^T  [C, T]
  - q^T, k^T computed as (W^T x^T) with j (head*dim) on partitions
  - rotate_half(q) computed on-chip as P @ q^T (signed permutation matmul)
  - v computed in natural [t, j] layout, augmented with a ones column so the
    attention-value matmul also produces the softmax denominator
  - scores computed transposed: s^T[k, q] = k^T(d,k)^T . q^T(d,q); softmax
    normalization deferred until after AV (flash-style), no max subtraction
    (scores are ~N(0,1); exp is safe in fp32)
  - output projection consumes y^T directly: out[t, c] = y^T(j,t)^T . Wp(j,c)
Matmul operands are bf16; softmax stays f32 in PSUM.

Perf notes (vs the first working version):
  - input DMAs fused (one per tensor) and ordered so the first projection
    matmul waits only on w1 + x-super-0 (~2 MB), not the full 6.5 MB preamble
  - scores computed in [128,1024] 2-bank PSUM pairs so one exp ACTIVATE
    covers two k-blocks (amortizes the ~230-cycle ACT fixed overhead)
  - causal masking: fully-masked prefix columns are skipped by narrowing the
    AV matmul range (no pt memsets), and the partially-masked boundary is a
    single 128-wide triangle multiply (not a full-width mask row)
  - softmax normalize is all on-chip: DVE reciprocal_approx_fast on the
    PSUM denominator row -> gpsimd partition_broadcast -> DVE multiply
    (replaces a 3-DMA DRAM bounce)
  - qkv bias-add moved from the ACT engine (which paces the attention inner
    loop with exps) to a DVE tensor_scalar add
  - RoPE cos/sin tables in bf16 (half the DMA bytes, 2x DVE rate on the mul)
  - projection / output-projection PE work is interleaved between attention
    score pairs so the in-order PE queue has fill work while exp runs
"""

import numpy as np

B, T, C, H, D = 2, 2048, 1024, 16, 64
G = 4           # heads per core
NCORES = 8
TS = 512        # t / q super-tile width
NT = T // 128   # 16 t-blocks
NTS = T // TS   # 4 t-supers

_cached = {}


def _apply_workarounds():
    """This neuronxcc build rejects TPB instructions with >1 embedded sem wait.
    Patch the Tile drain and add a BIR pass splitting extra waits into
    standalone EventSemaphore instructions on the same (in-order) engine."""
    import concourse.tile as tile
    import concourse.mybir as mybir
    from concourse.vector_clock import ScopedClock

    if getattr(tile.TileContext, "_multiwait_patched", False):
        return

    def _drain_and_barrier(self, tick_clock, wait_clock):
        nc = self.nc
        probe = nc.sync.nop(nofuse=True)
        wait_clock.add_sem_waits(probe.ins, ScopedClock({None: tick_clock.global_clock}))
        si = probe.ins.sync_info
        waits = list(si.on_wait) if si and si.on_wait else []
        if si is not None:
            si.on_wait = []
        by_num = {h.num: h for h in self.sems.allocated().values()}
        for w in waits:
            nc.sync.wait_ge(by_num[w.id], w.wait_value)
        nc.sync.drain()
        nc.all_engine_barrier()
        popped = nc._tile_sem_poison_stack.pop()
        assert popped is self._sem_poison
        nc.clear_and_free_semaphores(list(self.sems.allocated().values()))
        nc.all_engine_barrier()

    tile.TileContext._drain_and_barrier = _drain_and_barrier
    tile.TileContext._multiwait_patched = True


def _split_multiwaits(nc, maxw=1):
    import concourse.mybir as mybir

    n = 0
    for f in nc.m.functions:
        for bb in f.blocks:
            insts = list(bb.instructions)
            out = []
            changed = False
            for inst in insts:
                si = inst.sync_info
                waits = list(si.on_wait) if si and si.on_wait else []
                if len(waits) > maxw:
                    for k, w in enumerate(waits[: len(waits) - maxw]):
                        out.append(
                            mybir.InstEventSemaphore(
                                name=f"{inst.name}-xw{k}",
                                engine=inst.engine,
                                ins=[],
                                outs=[],
                                sync_info=mybir.SyncInfo(on_wait=[w], on_update=[]),
                            )
                        )
                        n += 1
                    si.on_wait = waits[len(waits) - maxw :]
                    changed = True
                out.append(inst)
            if changed:
                bb.instructions.clear()
                for i in out:
                    bb.add_instruction(i)
    return n


def _build():
    import concourse.bass as bass
    import concourse.mybir as mybir
    import concourse.tile as tile
    import concourse.bass_isa as bass_isa
    from concourse import library_config

    _apply_workarounds()

    f32 = mybir.dt.float32
    bf16 = mybir.dt.bfloat16
    Exp = mybir.ActivationFunctionType.Exp
    Ln = mybir.ActivationFunctionType.Ln

    nc = bass.Bass()

    # big inputs host-packed to SBUF layout: one fat contiguous descriptor
    # per partition row instead of 8 x 1KB scatter reads
    xT = nc.dram_tensor("xT", [NTS, 128, 8, TS], bf16, kind="ExternalInput")
    w1 = nc.dram_tensor("w1", [128, 8, 512], bf16, kind="ExternalInput")  # [q01 q23 k01 k23]
    b1 = nc.dram_tensor("b1", [128, 4], f32, kind="ExternalInput")
    wv = nc.dram_tensor("wv", [128, 8, 256], bf16, kind="ExternalInput")
    wp = nc.dram_tensor("wp", [128, 2, C], bf16, kind="ExternalInput")
    cosb = nc.dram_tensor("cosb", [128, T], bf16, kind="ExternalInput")
    sinb = nc.dram_tensor("sinb", [128, T], bf16, kind="ExternalInput")
    tri = nc.dram_tensor("tri", [128, 128], bf16, kind="ExternalInput")  # tri[k,q]=1 iff q>=k
    pt2 = nc.dram_tensor("pt2", [128, 128], bf16, kind="ExternalInput")  # rotate-half perm^T
    out = nc.dram_tensor("out", [T, C], f32, kind="ExternalOutput")

    with tile.TileContext(nc) as tc:
        with (
            tc.tile_pool(name="persist", bufs=1) as per,
            tc.tile_pool(name="xq", bufs=2) as xq,
            tc.tile_pool(name="qkr", bufs=4) as qkrp,
            tc.tile_pool(name="tmp", bufs=4) as tmp,
            tc.tile_pool(name="pp", bufs=4) as pp,
            tc.tile_pool(name="rr", bufs=3) as rr,
            tc.tile_pool(name="yo", bufs=4) as yo,
            tc.tile_pool(name="pairs", bufs=2, space="PSUM") as pairs_pool,
            tc.tile_pool(name="misc", bufs=2, space="PSUM") as misc,
            tc.tile_pool(name="psO", bufs=2, space="PSUM") as psO,
        ):
            # ---- persistent tiles ----
            w1_s = per.tile([128, 8, 512], bf16)
            wv_s = per.tile([128, 8, 256], bf16)
            wp_s = per.tile([128, 2, C], bf16)
            b1_s = per.tile([128, 4], f32)
            cos_s = per.tile([128, T], bf16)
            sin_s = per.tile([128, T], bf16)
            tri_s = per.tile([128, 128], bf16)
            pt2_s = per.tile([128, 128], bf16)
            qk_s = per.tile([128, 4, T], bf16)        # [q01' q23' k01' k23']
            # v storage per head pair:
            #   [v_even(0:64) | ones(64:66) | gap(66:97) | v_odd(97:161)]
            # The AV lhsT is a 128-wide window: even head -> cols 0..127, so y
            # lands in psum rows 0..63 with the denominator in row 64; odd
            # head -> cols 33..160, so y lands in rows 64..127 with the
            # denominator (ones col 65) in row 32. Engine ops require
            # 32-aligned partition starts, so denominator rows must be 32/64.
            v_s = per.tile([128, NT, 2, 161], bf16)
            yT_s = per.tile([128, 2, T], bf16)
            ones_s = per.tile([1, 64], bf16)       # 1/S broadcast stationary

            xt_tiles = {}

            def issue_xt(ts):
                xt = xq.tile([128, 8, TS], bf16, tag="xt")
                nc.sync.dma_start(out=xt, in_=xT[ts])
                xt_tiles[ts] = xt

            # DMA order: first-needed first, split across the two HWDGE
            # queues (sync + scalar) so the critical w1/x transfers aren't
            # stuck behind small-descriptor loads.  The first qk-projection
            # chain needs all of w1 + x super 0; cos/sin/b1 follow within the
            # first few us of compute; wv by ~8us; tri by the first
            # attention; wp only at the first output projection.
            nc.sync.dma_start(out=w1_s, in_=w1[:])
            issue_xt(0)
            nc.sync.dma_start(out=cos_s, in_=cosb[:])
            nc.sync.dma_start(out=sin_s, in_=sinb[:])
            nc.sync.dma_start(out=b1_s, in_=b1[:])
            nc.sync.dma_start(out=pt2_s, in_=pt2[:])
            nc.sync.dma_start(out=wv_s, in_=wv[:])
            nc.sync.dma_start(out=tri_s, in_=tri[:])
            nc.sync.dma_start(out=wp_s, in_=wp[:])
            # only the ones/gap region needs the 1.0 fill; the v copies
            # overwrite [0:64] and [97:161] every block
            nc.vector.memset(
                v_s[:, :, :, 64:97].rearrange("p a b c -> p (a b) c"), 1.0
            )
            nc.vector.memset(ones_s, 1.0)

            # ---- work units (closures emitted either inline or dripped
            # between attention score pairs to fill the in-order PE queue) ----

            # fill units are GENERATORS yielding after each matmul, so the
            # drip scheduler can interleave single matmuls between attention
            # score pairs without pushing the next scores (and therefore the
            # ACT exp stream) back by a whole projection chain
            def qk_unit(ts, jb):
                xt = xt_tiles[ts]
                tsl = slice(ts * TS, (ts + 1) * TS)
                ps = misc.tile([128, 512], f32, tag="mm")
                for cb in range(8):
                    nc.tensor.matmul(
                        ps,
                        w1_s[:, cb, jb * 128:(jb + 1) * 128],
                        xt[:, cb, :],
                        start=(cb == 0),
                        stop=(cb == 7),
                    )
                    yield
                qkr = qkrp.tile([128, TS], bf16, tag="qkr")
                nc.vector.tensor_scalar_add(qkr, ps, b1_s[:, jb:jb + 1])
                psr = misc.tile([128, 512], f32, tag="mm")
                nc.tensor.matmul(psr, pt2_s, qkr, start=True, stop=True)
                t1 = tmp.tile([128, TS], bf16, tag="t1")
                nc.vector.tensor_mul(t1, qkr, cos_s[:, tsl])
                t2 = tmp.tile([128, TS], bf16, tag="t2")
                nc.vector.tensor_mul(t2, psr, sin_s[:, tsl])
                nc.vector.tensor_add(qk_s[:, jb, tsl], t1, t2)
                yield

            def v_unit(ts, tb2):
                xt = xt_tiles[ts]
                tb = ts * 4 + tb2
                psv = misc.tile([128, 512], f32, tag="mm")
                for cb in range(8):
                    nc.tensor.matmul(
                        psv[:, :256],
                        xt[:, cb, tb2 * 128:(tb2 + 1) * 128],
                        wv_s[:, cb, :],
                        start=(cb == 0),
                        stop=(cb == 7),
                    )
                    if cb % 2:
                        yield
                psv4 = psv[:, :256].rearrange("p (pr par d) -> p pr par d", par=2, d=D)
                nc.vector.tensor_copy(v_s[:, tb, :, 0:64], psv4[:, :, 0, :])
                nc.vector.tensor_copy(v_s[:, tb, :, 97:161], psv4[:, :, 1, :])
                yield

            def oproj_unit(tb, cs):
                py = misc.tile([128, 512], f32, tag="mm")
                for jb in range(2):
                    nc.tensor.matmul(
                        py,
                        yT_s[:, jb, tb * 128:(tb + 1) * 128],
                        wp_s[:, jb, cs * 512:(cs + 1) * 512],
                        start=(jb == 0),
                        stop=(jb == 1),
                    )
                    yield
                ot = yo.tile([128, 512], f32, tag="ot")
                nc.vector.tensor_copy(ot, py)
                nc.sync.dma_start(
                    out=out[tb * 128:(tb + 1) * 128, cs * 512:(cs + 1) * 512],
                    in_=ot,
                )
                yield

            def xt_unit(ts):
                issue_xt(ts)
                yield

            def phase1_units(ts):
                us = [qk_unit(ts, jb) for jb in range(4)]
                us += [v_unit(ts, tb2) for tb2 in range(4)]
                return us

            def oproj_units(js):
                return [oproj_unit(js * 4 + tb2, cs) for tb2 in range(4) for cs in range(2)]

            # ---- attention for one q-super, dripping `fill` units between
            # score pairs so the PE never starves while ACT runs exps ----
            def do_attention(js, fill):
                qsl = slice(js * TS, (js + 1) * TS)
                nkb = 4 * js + 4
                npairs = nkb // 2

                def drip(n):
                    # emit up to n fill micro-ops (one matmul each)
                    while n and fill:
                        try:
                            next(fill[0])
                            n -= 1
                        except StopIteration:
                            fill.pop(0)

                for h in range(G):
                    par = h % 2
                    prow = slice(par * 64, par * 64 + 64)
                    srow = 64 - 32 * par  # denominator row (32-aligned)
                    qT = qk_s[prow, h // 2, :]
                    kT = qk_s[prow, 2 + h // 2, :]

                    def v_win(kb, pair=h // 2, par=par):
                        # 128-wide lhsT window into the [v_even |1|1| v_odd] slot
                        return v_s[:, kb, pair, 33 * par:33 * par + 128]

                    po = psO.tile([128, 512], f32, tag="po")
                    pend = []    # AV deferred 2 pairs behind scores/exp

                    def emit_av(p, po=po, v_win=v_win):
                        pt_, ws, pi_ = p
                        for half in (0, 1):
                            kb = 2 * pi_ + half
                            w0 = ws[half]
                            nc.tensor.matmul(
                                po[:, w0:] if w0 else po,
                                v_win(kb),
                                pt_[:, half * 512 + w0:half * 512 + 512],
                                start=(pi_ == 0 and half == 0),
                                stop=(pi_ == npairs - 1 and half == 1),
                            )

                    def emit_exp(sp, pi):
                        kb0, kb1 = 2 * pi, 2 * pi + 1
                        pt_ = pp.tile([128, 1024], bf16, tag="pt")
                        if pi < 2 * js:
                            # both k-blocks fully below the diagonal: one exp
                            # over the 2-bank pair
                            nc.scalar.activation(pt_, sp, Exp, scale=0.125)
                            ws = (0, 0)
                        else:
                            # diagonal super: prefix cols [0,w0) fully masked
                            # (skipped here AND in the AV range); boundary
                            # strip [w0,w0+128) masked with a triangle mul
                            ws = (128 * (kb0 - 4 * js), 128 * (kb1 - 4 * js))
                            for half in (0, 1):
                                base, w0 = half * 512, ws[half]
                                nc.scalar.activation(
                                    pt_[:, base + w0:base + 512],
                                    sp[:, base + w0:base + 512],
                                    Exp, scale=0.125,
                                )
                                nc.vector.tensor_mul(
                                    pt_[:, base + w0:base + w0 + 128],
                                    pt_[:, base + w0:base + w0 + 128],
                                    tri_s,
                                )
                        return (pt_, ws, pi)

                    # process pairs in quads: 4 score matmuls back-to-back,
                    # then 4 AVs — each row-group-mode transition on the PE
                    # costs ~85ns, so batch same-mode runs
                    for qi in range(npairs // 2):
                        sps = []
                        for pi in (2 * qi, 2 * qi + 1):
                            kb0, kb1 = 2 * pi, 2 * pi + 1
                            sp = pairs_pool.tile([128, 1024], f32, tag="sp")
                            nc.tensor.matmul(
                                sp[:, :512], kT[:, kb0 * 128:(kb0 + 1) * 128],
                                qT[:, qsl], start=True, stop=True,
                            )
                            nc.tensor.matmul(
                                sp[:, 512:], kT[:, kb1 * 128:(kb1 + 1) * 128],
                                qT[:, qsl], start=True, stop=True,
                            )
                            sps.append((sp, pi))
                        new = [emit_exp(sp, pi) for sp, pi in sps]
                        for p in pend:
                            emit_av(p)
                        pend = new
                        drip(2)
                    for p in pend:
                        emit_av(p)

                    # normalize y = po * (1/S), all on-chip and latency-short:
                    # 1/S = exp(-ln S) via two ACT ops straight off the PSUM
                    # denominator row (Ln and Exp share one ACT table), then
                    # a ones-stationary matmul broadcasts the [1,512] row
                    # across the 64 y partitions, and DVE multiplies.
                    lnS = rr.tile([1, 512], f32, tag="lnS")
                    nc.scalar.activation(lnS, po[srow:srow + 1, :], Ln)
                    inv = rr.tile([1, 512], bf16, tag="inv")
                    nc.scalar.activation(inv, lnS, Exp, scale=-1.0)
                    rb_ps = psO.tile([128, 512], f32, tag="po")
                    nc.tensor.matmul(rb_ps[prow, :], ones_s, inv, start=True, stop=True)
                    # TT may read only one PSUM operand -> bounce rb to SBUF
                    rbs = rr.tile([128, 512], bf16, tag="rbs")
                    nc.vector.tensor_copy(rbs[prow, :], rb_ps[prow, :])
                    nc.vector.tensor_mul(yT_s[prow, h // 2, qsl], po[prow, :], rbs[prow, :])
                    # the normalize chain has ~2.5us of cross-engine latency;
                    # burst fill so the PE stays busy across it
                    drip(8)

                while fill:
                    try:
                        next(fill[0])
                    except StopIteration:
                        fill.pop(0)

            # ---- main schedule ----
            for g in phase1_units(0):
                for _ in g:
                    pass
            for ts in range(NTS):
                fill = []
                if ts + 1 < NTS:
                    fill.append(xt_unit(ts + 1))
                if ts >= 1:
                    fill += oproj_units(ts - 1)
                if ts + 1 < NTS:
                    fill += phase1_units(ts + 1)
                do_attention(ts, fill)
            for g in oproj_units(NTS - 1):
                for _ in g:
                    pass

    _split_multiwaits(nc)
    return nc


def _host_inputs(x, W_attn, b_attn, W_proj):
    f32 = np.float32
    import ml_dtypes

    bf16 = ml_dtypes.bfloat16

    inv = (1.0 / (10000.0 ** (np.arange(0, D, 2, dtype=f32) / f32(D)))).astype(f32)
    t = np.arange(T, dtype=f32)
    ang = np.outer(inv, t).astype(f32)            # [32, T]
    cos32, sin32 = np.cos(ang).astype(f32), np.sin(ang).astype(f32)
    cosb = np.tile(cos32, (4, 1)).astype(bf16)     # [128, T], row p -> freq p%32
    sinb = np.tile(sin32, (4, 1)).astype(bf16)

    kk = np.arange(128)[:, None]
    qq = np.arange(128)[None, :]
    tri = np.where(qq >= kk, f32(1), f32(0)).astype(bf16)  # [128,128]

    p64 = np.zeros((D, D), dtype=f32)
    for d in range(32):
        p64[d, d + 32] = -1.0
        p64[d + 32, d] = 1.0
    pt2 = np.zeros((128, 128), dtype=f32)
    pt2[:64, :64] = p64.T
    pt2[64:, 64:] = p64.T
    pt2 = pt2.astype(bf16)

    def pack(m):
        # [8*128, cols] -> [128, 8, cols]: SBUF chunk layout, contiguous per
        # partition row so each DMA descriptor is one fat read
        cols = m.shape[1]
        return np.ascontiguousarray(m.reshape(8, 128, cols).transpose(1, 0, 2))

    # x[b] [T, C] -> [NTS, 128(part=c%128... c-chunks), 8, TS]
    xTs = [
        np.ascontiguousarray(
            x[b].reshape(NTS, TS, 8, 128).transpose(0, 3, 2, 1)
        ).astype(bf16)
        for b in range(B)
    ]

    per_g = []
    for g in range(G):
        hs = [4 * g + j for j in range(G)]
        qcols = [W_attn[:, h * D:(h + 1) * D] for h in hs]
        kcols = [W_attn[:, C + h * D:C + (h + 1) * D] for h in hs]
        qb = [b_attn[h * D:(h + 1) * D] for h in hs]
        kb_ = [b_attn[C + h * D:C + (h + 1) * D] for h in hs]
        w1 = pack(np.concatenate(
            [qcols[0], qcols[1], qcols[2], qcols[3], kcols[0], kcols[1], kcols[2], kcols[3]],
            axis=1,
        ).astype(bf16))                            # [128, 8, 512]: [q01 q23 k01 k23]
        b1 = np.concatenate(qb + kb_).astype(f32).reshape(4, 128).T.copy()  # [128, 4]
        wv_ = pack(W_attn[:, 2 * C + 256 * g:2 * C + 256 * (g + 1)].astype(bf16))
        wp_g = W_proj[256 * g:256 * (g + 1), :].astype(bf16)   # [256, C]
        wp_ = np.ascontiguousarray(wp_g.reshape(2, 128, C).transpose(1, 0, 2))
        per_g.append((w1, b1, wv_, wp_))

    shared = dict(cosb=cosb, sinb=sinb, tri=tri, pt2=pt2)
    in_maps = []
    for i in range(NCORES):
        b, g = i // 4, i % 4
        w1, b1, wv_, wp_ = per_g[g]
        in_maps.append(dict(xT=xTs[b], w1=w1, b1=b1, wv=wv_, wp=wp_, **shared))
    return in_maps


def kernel(x, W_attn, b_attn, W_proj, b_proj):
    from concourse.bass_utils import run_bass_kernel_spmd

    x = np.asarray(x, dtype=np.float32)
    W_attn = np.asarray(W_attn, dtype=np.float32)
    b_attn = np.asarray(b_attn, dtype=np.float32)
    W_proj = np.asarray(W_proj, dtype=np.float32)
    b_proj = np.asarray(b_proj, dtype=np.float32)

    if "nc" not in _cached:
        _cached["nc"] = _build()
    nc = _cached["nc"]

    in_maps = _host_inputs(x, W_attn, b_attn, W_proj)
    res = run_bass_kernel_spmd(nc, in_maps, core_ids=list(range(NCORES)))
    _cached["last_results"] = res

    const = (b_proj + b_attn[2 * C:] @ W_proj).astype(np.float32)
    y = np.empty((B, T, C), dtype=np.float32)
    for b in range(B):
        acc = res.results[4 * b]["out"].astype(np.float32).copy()
        for g in range(1, 4):
            acc += res.results[4 * b + g]["out"]
        y[b] = acc + const
    return y
